# revision 6
# baseline (speedup 1.0000x reference)
"""VSE-style contrastive hinge loss + similarity matrix on 8 TRN2 NeuronCores.

reference:
    im_n = l2norm(image_features); tx_n = l2norm(text_features)
    sim  = im_n @ tx_n.T                       [B, B]
    loss = sum over i != j of relu(M + sim[i,j] - sim[i,i])
                            + relu(M + sim[i,j] - sim[j,j])
    returns (loss, sim)

Sharding: rows of `im` are split 512/core; every core holds the full `tx`
(transposed for the matmul contraction). Each core computes its [512, 4096]
sim block plus a partial loss. Per-shard inverse tx-norms and diagonal values
are exchanged with one 4 KB AllGather so each core can apply the column-wise
hinge bias. Host work is only slicing/transposing inputs and concatenating
the outputs.
"""

import sys

sys.path.insert(0, "/opt/trn_rl_repo")

import numpy as np

import concourse.bass as bass
import concourse.bacc as bacc
import concourse.mybir as mybir
import concourse.tile as tile

B = 4096
D = 1024
W = 8
MARGIN = 0.2
EPS = 1e-12

F32 = mybir.dt.float32
BF16 = mybir.dt.bfloat16
AF = mybir.ActivationFunctionType
ALU = mybir.AluOpType


def build_kernel(BB=B, DD=D, WW=W):
    """Build the SPMD Bacc program (identical on all WW cores)."""
    R = BB // WW          # rows per core
    KT = DD // 128        # contraction k-tiles
    TT = R // 128         # 128-row tiles per core
    NCH = BB // 512       # 512-col psum chunks
    assert R % 128 == 0 and DD % 128 == 0 and BB % 512 == 0

    nc = bacc.Bacc("TRN2", target_bir_lowering=False, debug=False,
                   num_devices=WW)

    imT = nc.dram_tensor("imT", [DD, R], F32, kind="ExternalInput")
    txT = nc.dram_tensor("txT", [DD, BB], F32, kind="ExternalInput")
    txTs = nc.dram_tensor("txTs", [DD, R], F32, kind="ExternalInput")
    sim_out = nc.dram_tensor("sim", [R, BB], F32, kind="ExternalOutput")
    loss_out = nc.dram_tensor("loss", [1, 1], F32, kind="ExternalOutput")

    with tile.TileContext(nc) as tc:
        with (
            tc.tile_pool(name="res", bufs=1) as res,
            tc.tile_pool(name="stage", bufs=2) as stage,
            tc.tile_pool(name="stats", bufs=2) as stats,
            tc.tile_pool(name="dram", bufs=1, space="DRAM") as dram,
        ):
            ones_bf = res.tile([128, 1], BF16, name="ones_bf")
            nc.vector.memset(ones_bf[:], 1.0)
            ones_f = res.tile([128, 1], F32, name="ones_f")
            nc.vector.memset(ones_f[:], 1.0)

            # ---- load + cast to bf16 --------------------------------------
            tx_bf = res.tile([128, KT, BB], BF16, name="tx_bf")
            im_bf = res.tile([128, KT, R], BF16, name="im_bf")
            txs_bf = res.tile([128, KT, R], BF16, name="txs_bf")
            for kt in range(KT):
                t_st = stage.tile([128, BB], F32, name="t_st")
                nc.sync.dma_start(t_st[:], txT[kt * 128:(kt + 1) * 128, :])
                if kt % 2 == 0:
                    nc.vector.tensor_copy(tx_bf[:, kt, :], t_st[:])
                else:
                    nc.scalar.copy(tx_bf[:, kt, :], t_st[:])
            for kt in range(KT):
                i_st = stage.tile([128, R], F32, name="i_st")
                nc.sync.dma_start(i_st[:], imT[kt * 128:(kt + 1) * 128, :])
                nc.vector.tensor_copy(im_bf[:, kt, :], i_st[:])
                s_st = stage.tile([128, R], F32, name="s_st")
                nc.sync.dma_start(s_st[:], txTs[kt * 128:(kt + 1) * 128, :])
                nc.vector.tensor_copy(txs_bf[:, kt, :], s_st[:])

            # ---- per-shard stats: sum over d of im^2, tx^2, im*tx ---------
            with tc.tile_pool(name="spsum", bufs=1, space="PSUM") as spsum:
                ps_imn = spsum.tile([1, R], F32, name="ps_imn")
                ps_txn = spsum.tile([1, R], F32, name="ps_txn")
                ps_dg = spsum.tile([1, R], F32, name="ps_dg")
                for kt in range(KT):
                    sq_i = stats.tile([128, R], BF16, name="sq_i")
                    nc.scalar.activation(sq_i[:], im_bf[:, kt, :], AF.Square)
                    nc.tensor.matmul(ps_imn[:], ones_bf[:], sq_i[:],
                                     start=(kt == 0), stop=(kt == KT - 1))
                    sq_t = stats.tile([128, R], BF16, name="sq_t")
                    nc.scalar.activation(sq_t[:], txs_bf[:, kt, :], AF.Square)
                    nc.tensor.matmul(ps_txn[:], ones_bf[:], sq_t[:],
                                     start=(kt == 0), stop=(kt == KT - 1))
                    pr = stats.tile([128, R], BF16, name="pr")
                    nc.vector.tensor_mul(pr[:], im_bf[:, kt, :],
                                         txs_bf[:, kt, :])
                    nc.tensor.matmul(ps_dg[:], ones_bf[:], pr[:],
                                     start=(kt == 0), stop=(kt == KT - 1))

                # norms -> inverse norms; diag = (im.tx) * inv_im * inv_tx
                inv_im = res.tile([1, R], F32, name="inv_im")
                nc.scalar.activation(inv_im[:], ps_imn[:], AF.Sqrt)
                nc.vector.tensor_scalar_max(inv_im[:], inv_im[:], EPS)
                nc.vector.reciprocal(inv_im[:], inv_im[:])

                inv_tx = res.tile([1, R], F32, name="inv_tx")
                nc.scalar.activation(inv_tx[:], ps_txn[:], AF.Sqrt)
                nc.vector.tensor_scalar_max(inv_tx[:], inv_tx[:], EPS)
                nc.vector.reciprocal(inv_tx[:], inv_tx[:])

                d_loc = res.tile([1, R], F32, name="d_loc")
                nc.scalar.copy(d_loc[:], ps_dg[:])
                nc.vector.tensor_mul(d_loc[:], d_loc[:], inv_im[:])
                nc.vector.tensor_mul(d_loc[:], d_loc[:], inv_tx[:])

            # mbias_row = MARGIN - d (for the row-wise hinge bias)
            mb_row = res.tile([1, R], F32, name="mb_row")
            nc.scalar.activation(mb_row[:], d_loc[:], AF.Copy,
                                 bias=MARGIN, scale=-1.0)

            # ---- AllGather {inv_tx, diag} across the 8 cores --------------
            ag_in = dram.tile([1, 2 * R], F32, name="ag_in")
            ag_out = dram.tile([WW, 2 * R], F32, name="ag_out",
                               addr_space="Shared")
            nc.sync.dma_start(ag_in[0:1, 0:R], inv_tx[:])
            nc.sync.dma_start(ag_in[0:1, R:2 * R], d_loc[:])
            nc.gpsimd.collective_compute(
                "AllGather", ALU.bypass,
                replica_groups=[list(range(WW))],
                ins=[ag_in.opt()], outs=[ag_out.opt()],
            )

            # broadcast gathered rows onto all 128 partitions
            invtx_bc = res.tile([128, WW, R], F32, name="invtx_bc")
            nc.sync.dma_start(
                invtx_bc[:],
                ag_out[:, 0:R].unsqueeze(0).to_broadcast((128, WW, R)))
            c2bc = res.tile([128, WW, R], F32, name="c2bc")
            nc.sync.dma_start(
                c2bc[:],
                ag_out[:, R:2 * R].unsqueeze(0).to_broadcast((128, WW, R)))
            # c2bc = MARGIN - d_j  (column-wise hinge bias)
            c2f = c2bc[:].rearrange("p w r -> p (w r)")
            nc.scalar.activation(c2f, c2f, AF.Copy, bias=MARGIN, scale=-1.0)
            zbias = res.tile([128, 1], F32, name="zbias")
            nc.vector.memset(zbias[:], 0.0)

            # per-row-tile bias/scale columns via a DRAM bounce
            scr = dram.tile([1, 2 * R], F32, name="scr")
            nc.sync.dma_start(scr[0:1, 0:R], mb_row[:])
            nc.sync.dma_start(scr[0:1, R:2 * R], inv_im[:])
            mb_col = res.tile([128, TT], F32, name="mb_col")
            nc.sync.dma_start(
                mb_col[:], scr[0:1, 0:R].rearrange("o (t p) -> (o p) t", p=128))
            invim_col = res.tile([128, TT], F32, name="invim_col")
            nc.sync.dma_start(
                invim_col[:],
                scr[0:1, R:2 * R].rearrange("o (t p) -> (o p) t", p=128))

            # ---- main loop: sim block + fused hinge-loss accumulation ----
            slots1 = res.tile([128, TT * NCH], F32, name="slots1")
            slots2 = res.tile([128, TT * NCH], F32, name="slots2")
            invtx_f = invtx_bc[:].rearrange("p w r -> p (w r)")
            with (
                tc.tile_pool(name="mpsum", bufs=8, space="PSUM") as mpsum,
                tc.tile_pool(name="mch", bufs=4) as mch,
                tc.tile_pool(name="mscr", bufs=2) as mscr,
            ):
                for t in range(TT):
                    pss = []
                    for n in range(NCH):
                        ps = mpsum.tile([128, 512], F32, name="ps")
                        pss.append(ps)
                    for kt in range(KT):
                        for n in range(NCH):
                            nc.tensor.matmul(
                                pss[n][:],
                                im_bf[:, kt, t * 128:(t + 1) * 128],
                                tx_bf[:, kt, n * 512:(n + 1) * 512],
                                start=(kt == 0), stop=(kt == KT - 1))
                    for n in range(NCH):
                        idx = t * NCH + n
                        simf = mch.tile([128, 512], F32, name="simf")
                        # row scale while evicting PSUM
                        nc.scalar.activation(simf[:], pss[n][:], AF.Copy,
                                             scale=invim_col[:, t:t + 1])
                        # column scale -> final sim values
                        nc.vector.tensor_mul(
                            simf[:], simf[:],
                            invtx_f[:, n * 512:(n + 1) * 512])
                        nc.sync.dma_start(
                            sim_out[t * 128:(t + 1) * 128,
                                    n * 512:(n + 1) * 512], simf[:])
                        # caption-negative hinge: relu(sim + (M - d_i)), summed
                        sc1 = mscr.tile([128, 512], BF16, name="sc1")
                        nc.scalar.activation(
                            sc1[:], simf[:], AF.Relu,
                            bias=mb_col[:, t:t + 1],
                            accum_out=slots1[:, idx:idx + 1])
                        # image-negative hinge: relu(sim + (M - d_j)), summed
                        u2 = mscr.tile([128, 512], BF16, name="u2")
                        nc.vector.tensor_add(
                            u2[:], simf[:], c2f[:, n * 512:(n + 1) * 512])
                        sc2 = mscr.tile([128, 512], BF16, name="sc2")
                        nc.scalar.activation(
                            sc2[:], u2[:], AF.Relu, bias=zbias[:],
                            accum_out=slots2[:, idx:idx + 1])

            # ---- loss reduction ------------------------------------------
            red1 = res.tile([128, 1], F32, name="red1")
            nc.vector.reduce_sum(red1[:], slots1[:],
                                 axis=mybir.AxisListType.X)
            red2 = res.tile([128, 1], F32, name="red2")
            nc.vector.reduce_sum(red2[:], slots2[:],
                                 axis=mybir.AxisListType.X)
            nc.vector.tensor_add(red1[:], red1[:], red2[:])
            with tc.tile_pool(name="lpsum", bufs=1, space="PSUM") as lpsum:
                ps_l = lpsum.tile([1, 1], F32, name="ps_l")
                nc.tensor.matmul(ps_l[:], red1[:], ones_f[:],
                                 start=True, stop=True)
                lf = res.tile([1, 1], F32, name="lf")
                nc.scalar.copy(lf[:], ps_l[:])
            # subtract the diagonal (i == j) terms: 2*MARGIN per own row
            nc.vector.tensor_scalar_sub(lf[:], lf[:],
                                        float(2.0 * R * MARGIN))
            nc.sync.dma_start(loss_out[0:1, 0:1], lf[:])

    nc.compile()
    return nc


_NC_CACHE = {}


def _get_nc(BB=B, DD=D, WW=W):
    key = (BB, DD, WW)
    if key not in _NC_CACHE:
        _NC_CACHE[key] = build_kernel(BB, DD, WW)
    return _NC_CACHE[key]


def make_in_maps(im, tx, WW=W):
    BB = im.shape[0]
    R = BB // WW
    txT = np.ascontiguousarray(tx.T)
    maps = []
    for c in range(WW):
        maps.append({
            "imT": np.ascontiguousarray(im[c * R:(c + 1) * R].T),
            "txT": txT,
            "txTs": np.ascontiguousarray(tx[c * R:(c + 1) * R].T),
        })
    return maps


def kernel(image_features, text_features):
    from concourse.bass_utils import run_bass_kernel_spmd

    im = np.ascontiguousarray(np.asarray(image_features, dtype=np.float32))
    tx = np.ascontiguousarray(np.asarray(text_features, dtype=np.float32))
    nc = _get_nc(im.shape[0], im.shape[1], W)
    res = run_bass_kernel_spmd(nc, make_in_maps(im, tx, W),
                               core_ids=list(range(W)))
    sim = np.concatenate([r["sim"] for r in res.results], axis=0)
    loss = np.float32(sum(float(r["loss"][0, 0]) for r in res.results))
    return loss, sim


# revision 7
# speedup vs baseline: 1.0764x; 1.0764x over previous
"""VSE contrastive hinge loss + similarity matrix on 8 TRN2 NeuronCores.

Two-NEFF pipeline (collectives cost ~80us of barrier/trigger overhead in this
environment, measured, so the cross-core exchange goes through the host):

  Kernel A (stats): core c owns row-block c (512 rows) of BOTH feature
  matrices; computes inv_norm(im), inv_norm(tx) and the diagonal
  d = cos(im_c, tx_c); outputs {inv_im, inv_tx, M-d, d-M} in a [128, 16]
  column-tile layout (out[p, 4*g + t] = vec_g[t*128 + p]).

  Host: concatenates the 8 stat blocks and re-slices them per kernel-B core
  (pure data movement).

  Kernel B (main): 4x2 grid - core c=(r,s) owns im rows I_r (1024) x tx rows
  J_s (2048). Computes raw im @ tx^T with bf16 matmuls (tx pre-scaled by
  inv_tx during the f32->bf16 cast), row-scales by inv_im on PSUM eviction,
  writes its [1024, 2048] f32 sim block, and accumulates both hinge sums:
    r1 = sum relu(sim + (M - d_i))     (ACT relu + accum, per-row bias)
    r2 = sum max(sim, d_j - M)         (DVE max + reduce, corrected by
                                        - rows * sum_j (d_j - M))
  minus 2*M*512 per core for the diagonal cells (summed over the 8 cores
  this equals the reference's diagonal mask up to fp rounding).
"""

import sys

sys.path.insert(0, "/opt/trn_rl_repo")

import ml_dtypes
import numpy as np

import concourse.bass as bass
import concourse.bacc as bacc
import concourse.mybir as mybir
import concourse.tile as tile

B = 4096
D = 1024
W = 8
GR = 4            # im row blocks (kernel B grid)
GS = 2            # tx row blocks
MARGIN = 0.2
EPS = 1e-12

F32 = mybir.dt.float32
BF16 = mybir.dt.bfloat16
AF = mybir.ActivationFunctionType
ALU = mybir.AluOpType
AX = mybir.AxisListType


def build_stats(BB=B, DD=D, WW=W):
    """Kernel A: per-shard norms + diagonal, outputs in [128, 4G] layout."""
    R = BB // WW          # 512
    KT = DD // 128        # 8
    TT = R // 128         # 4 columns per quantity

    nc = bacc.Bacc("TRN2", target_bir_lowering=False, debug=False,
                   num_devices=WW)
    imTs = nc.dram_tensor("imTs", [DD, R], F32, kind="ExternalInput")
    txTs = nc.dram_tensor("txTs", [DD, R], F32, kind="ExternalInput")
    stats_out = nc.dram_tensor("stats", [128, 4 * TT], F32,
                               kind="ExternalOutput")

    with tile.TileContext(nc) as tc:
        with (
            tc.tile_pool(name="res", bufs=1) as res,
            tc.tile_pool(name="stage", bufs=2) as stage,
            tc.tile_pool(name="scr", bufs=3) as scr,
            tc.tile_pool(name="dram", bufs=1, space="DRAM") as dram,
            tc.tile_pool(name="ppool", bufs=1, space="PSUM") as ppool,
        ):
            ones_bf = res.tile([128, 1], BF16, name="ones_bf")
            nc.vector.memset(ones_bf[:], 1.0)

            # two halves per tensor -> 4 big DMAs total
            HK = KT // 2
            im_st = [stage.tile([128, HK, R], F32, name="im_st", tag="ist")
                     for _ in range(2)]
            tx_st = [stage.tile([128, HK, R], F32, name="tx_st", tag="tst")
                     for _ in range(2)]
            for h in range(2):
                src = imTs[h * HK * 128:(h + 1) * HK * 128, :]
                nc.sync.dma_start(
                    im_st[h][:], src.rearrange("(k p) r -> p k r", p=128))
                src = txTs[h * HK * 128:(h + 1) * HK * 128, :]
                nc.sync.dma_start(
                    tx_st[h][:], src.rearrange("(k p) r -> p k r", p=128))

            ps_imn = ppool.tile([1, R], F32, name="ps_imn")
            ps_txn = ppool.tile([1, R], F32, name="ps_txn")
            ps_dg = ppool.tile([1, R], F32, name="ps_dg")
            for kt in range(KT):
                h, k = divmod(kt, HK)
                ims = im_st[h][:, k, :]
                txs = tx_st[h][:, k, :]
                sq_i = scr.tile([128, R], BF16, name="sq_i")
                nc.scalar.activation(sq_i[:], ims, AF.Square)
                nc.tensor.matmul(ps_imn[:], ones_bf[:], sq_i[:],
                                 start=(kt == 0), stop=(kt == KT - 1))
                sq_t = scr.tile([128, R], BF16, name="sq_t")
                nc.vector.tensor_mul(sq_t[:], txs, txs)
                nc.tensor.matmul(ps_txn[:], ones_bf[:], sq_t[:],
                                 start=(kt == 0), stop=(kt == KT - 1))
                pr = scr.tile([128, R], BF16, name="pr")
                nc.vector.tensor_mul(pr[:], ims, txs)
                nc.tensor.matmul(ps_dg[:], ones_bf[:], pr[:],
                                 start=(kt == 0), stop=(kt == KT - 1))

            # evict the three [1, R] rows, bounce through DRAM into a
            # [128, 3*TT] column layout so the inverse chain runs wide
            row3 = res.tile([1, 3 * R], F32, name="row3")
            nc.scalar.copy(row3[0:1, 0:R], ps_imn[:])
            nc.scalar.copy(row3[0:1, R:2 * R], ps_txn[:])
            nc.scalar.copy(row3[0:1, 2 * R:3 * R], ps_dg[:])
            bounce = dram.tile([1, 3 * R], F32, name="bounce")
            nc.sync.dma_start(bounce[:], row3[:])
            nst = res.tile([128, 3 * TT], F32, name="nst")
            nc.sync.dma_start(
                nst[:], bounce[0:1, :].rearrange("o (g t p) -> (o p) (g t)",
                                                 p=128, g=3))

            # cols [0:TT]=sum im^2, [TT:2TT]=sum tx^2, [2TT:3TT]=im.tx
            nrm = nst[:, 0:2 * TT]
            nc.scalar.activation(nrm, nrm, AF.Sqrt)
            nc.vector.tensor_scalar_max(nrm, nrm, EPS)
            nc.vector.reciprocal(nrm, nrm)
            dg = nst[:, 2 * TT:3 * TT]
            nc.vector.tensor_mul(dg, dg, nst[:, 0:TT])
            nc.vector.tensor_mul(dg, dg, nst[:, TT:2 * TT])

            out_t = res.tile([128, 4 * TT], F32, name="out_t")
            nc.vector.tensor_copy(out_t[:, 0:2 * TT], nst[:, 0:2 * TT])
            # M - d and d - M
            nc.scalar.activation(out_t[:, 2 * TT:3 * TT], dg, AF.Copy,
                                 bias=MARGIN, scale=-1.0)
            nc.vector.tensor_scalar_sub(out_t[:, 3 * TT:4 * TT], dg, MARGIN)
            nc.sync.dma_start(stats_out[:], out_t[:])

    nc.compile()
    return nc


def build_main(BB=B, DD=D):
    """Kernel B: sim block + hinge loss on the 4x2 grid."""
    RI = BB // GR          # 1024 im rows per core
    RJ = BB // GS          # 2048 tx rows per core
    KT = DD // 128         # 8
    TT = RI // 128         # 8
    NCH = RJ // 512        # 4

    nc = bacc.Bacc("TRN2", target_bir_lowering=False, debug=False,
                   num_devices=GR * GS)
    imT = nc.dram_tensor("imT", [DD, RI], F32, kind="ExternalInput")
    txT = nc.dram_tensor("txT", [DD, RJ], F32, kind="ExternalInput")
    invim_col = nc.dram_tensor("invim_col", [128, TT], F32,
                               kind="ExternalInput")
    mb_col = nc.dram_tensor("mb_col", [128, TT], F32, kind="ExternalInput")
    invtx_row = nc.dram_tensor("invtx_row", [1, RJ], BF16,
                               kind="ExternalInput")
    nc2_row = nc.dram_tensor("nc2_row", [1, RJ], BF16, kind="ExternalInput")
    sim_out = nc.dram_tensor("sim", [RI, RJ], F32, kind="ExternalOutput")
    loss_out = nc.dram_tensor("loss", [1, 1], F32, kind="ExternalOutput")

    with tile.TileContext(nc) as tc:
        with (
            tc.tile_pool(name="res", bufs=1) as res,
            tc.tile_pool(name="stage", bufs=2) as stage,
            tc.tile_pool(name="mch", bufs=4) as mch,
            tc.tile_pool(name="mscr", bufs=3) as mscr,
        ):
            # order matters: the matmul-feeding loads go first, the
            # loss-only tensors (mbc, nc2bc) after them
            ivc = res.tile([128, TT], F32, name="ivc")
            nc.sync.dma_start(ivc[:], invim_col[:])
            invtx_bc = res.tile([128, RJ], BF16, name="invtx_bc")
            nc.sync.dma_start(invtx_bc[:],
                              invtx_row[0:1, :].to_broadcast((128, RJ)))

            # loads: im as 2 big DMAs, tx as 4; casts consume stage slices.
            # tx gets pre-scaled by inv_tx during the cast.
            im_bf = res.tile([128, KT, RI], BF16, name="im_bf")
            tx_bf = res.tile([128, KT, RJ], BF16, name="tx_bf")
            HK = KT // 2
            im_st = [stage.tile([128, HK, RI], F32, name="im_st", tag="ist")
                     for _ in range(2)]
            tx_st = [stage.tile([128, RJ], F32, name="tx_st", tag="tst",
                                bufs=3)
                     for _ in range(KT)]
            nc.sync.dma_start(tx_st[0][:], txT[0:128, :])
            nc.sync.dma_start(tx_st[1][:], txT[128:256, :])
            nc.sync.dma_start(
                im_st[0][:],
                imT[0:HK * 128, :].rearrange("(k p) r -> p k r", p=128))
            nc.sync.dma_start(tx_st[2][:], txT[256:384, :])
            nc.sync.dma_start(tx_st[3][:], txT[384:512, :])
            nc.sync.dma_start(
                im_st[1][:],
                imT[HK * 128:KT * 128, :].rearrange("(k p) r -> p k r",
                                                    p=128))
            for kt in range(4, KT):
                nc.sync.dma_start(tx_st[kt][:],
                                  txT[kt * 128:(kt + 1) * 128, :])

            mbc = res.tile([128, TT], F32, name="mbc")
            nc.sync.dma_start(mbc[:], mb_col[:])
            nc2bc = res.tile([128, RJ], BF16, name="nc2bc")
            nc.sync.dma_start(nc2bc[:],
                              nc2_row[0:1, :].to_broadcast((128, RJ)))
            cs = res.tile([1, 1], F32, name="cs")
            nc.vector.reduce_sum(cs[:], nc2bc[0:1, :], axis=AX.X)
            ones_f = res.tile([128, 1], F32, name="ones_f")
            nc.vector.memset(ones_f[:], 1.0)
            zbias = res.tile([128, 1], F32, name="zbias")
            nc.vector.memset(zbias[:], 0.0)

            for kt in range(KT):
                nc.vector.tensor_mul(tx_bf[:, kt, :], tx_st[kt][:],
                                     invtx_bc[:])
            for kt in range(KT):
                h, k = divmod(kt, HK)
                nc.scalar.copy(im_bf[:, kt, :], im_st[h][:, k, :])

            slots1 = res.tile([128, TT], F32, name="slots1")
            slots2 = res.tile([128, TT], F32, name="slots2")

            with tc.tile_pool(name="mpsum", bufs=8, space="PSUM") as mpsum:
                for t in range(TT):
                    pss = [mpsum.tile([128, 512], F32, name="ps")
                           for _ in range(NCH)]
                    for kt in range(KT):
                        for n in range(NCH):
                            nc.tensor.matmul(
                                pss[n][:],
                                im_bf[:, kt, t * 128:(t + 1) * 128],
                                tx_bf[:, kt, n * 512:(n + 1) * 512],
                                start=(kt == 0), stop=(kt == KT - 1))
                    simf = mch.tile([128, RJ], F32, name="simf")
                    for n in range(NCH):
                        dst = simf[:, n * 512:(n + 1) * 512]
                        if n % 2 == 0:
                            nc.scalar.activation(dst, pss[n][:], AF.Copy,
                                                 scale=ivc[:, t:t + 1])
                        else:
                            nc.vector.tensor_scalar_mul(dst, pss[n][:],
                                                        ivc[:, t:t + 1])
                    nc.gpsimd.dma_start(sim_out[t * 128:(t + 1) * 128, :],
                                        simf[:])
                    # caption-negative hinge
                    sc1 = mscr.tile([128, RJ], BF16, name="sc1")
                    nc.scalar.activation(sc1[:], simf[:], AF.Relu,
                                         bias=mbc[:, t:t + 1],
                                         accum_out=slots1[:, t:t + 1])
                    # image-negative hinge via max(sim, d_j - M); the row
                    # sum alternates DVE reduce / ACT accumulate to balance
                    mx = mscr.tile([128, RJ], BF16, name="mx")
                    nc.vector.tensor_max(mx[:], simf[:], nc2bc[:])
                    if t % 2 == 0:
                        nc.vector.reduce_sum(slots2[:, t:t + 1], mx[:],
                                             axis=AX.X)
                    else:
                        jk = mscr.tile([128, RJ], BF16, name="jk")
                        nc.scalar.activation(jk[:], mx[:], AF.Identity,
                                             bias=zbias[:],
                                             accum_out=slots2[:, t:t + 1])

            red1 = res.tile([128, 1], F32, name="red1")
            nc.vector.reduce_sum(red1[:], slots1[:], axis=AX.X)
            red2 = res.tile([128, 1], F32, name="red2")
            nc.vector.reduce_sum(red2[:], slots2[:], axis=AX.X)
            nc.vector.tensor_add(red1[:], red1[:], red2[:])
            lf = res.tile([1, 1], F32, name="lf")
            with tc.tile_pool(name="lpsum", bufs=1, space="PSUM") as lpsum:
                ps_l = lpsum.tile([1, 1], F32, name="ps_l")
                nc.tensor.matmul(ps_l[:], red1[:], ones_f[:], start=True,
                                 stop=True)
                nc.scalar.copy(lf[:], ps_l[:])
            # r2 correction: subtract RI * sum_j (d_j - M)
            nc.vector.tensor_scalar_mul(cs[:], cs[:], float(RI))
            nc.vector.tensor_sub(lf[:], lf[:], cs[:])
            # diagonal cells: minus 2*M*(B/8) per core, summing to 2*M*B
            nc.vector.tensor_scalar_sub(
                lf[:], lf[:], float(2.0 * MARGIN * BB / (GR * GS)))
            nc.sync.dma_start(loss_out[0:1, 0:1], lf[:])

    nc.compile()
    return nc


_CACHE = {}

# test-harness knobs: when TRACE is set, each NEFF run is profiled and its
# exec_time_ns is appended to LAST_EXEC_NS
TRACE = False
LAST_EXEC_NS = []


def _get(which, *args):
    key = (which, args)
    if key not in _CACHE:
        _CACHE[key] = (build_stats(*args) if which == "A"
                       else build_main(*args))
    return _CACHE[key]


def _run(nc, maps, tag):
    from concourse.bass_utils import run_bass_kernel_spmd

    if not TRACE:
        return run_bass_kernel_spmd(nc, maps, core_ids=list(range(len(maps))))
    import os
    import shutil
    tmpdir = f"/tmp/trn_trace_{tag}"
    shutil.rmtree(tmpdir, ignore_errors=True)
    os.makedirs(tmpdir)
    res = run_bass_kernel_spmd(nc, maps, core_ids=list(range(len(maps))),
                               trace=True, tmpdir=tmpdir)
    LAST_EXEC_NS.append((tag, res.exec_time_ns, tmpdir))
    return res


def _decode_stats_block(block):
    """[128, 4*TT] column-tile layout -> four [R] vectors."""
    TT = block.shape[1] // 4
    return [block[:, g * TT:(g + 1) * TT].T.reshape(-1) for g in range(4)]


def kernel(image_features, text_features):
    im = np.ascontiguousarray(np.asarray(image_features, dtype=np.float32))
    tx = np.ascontiguousarray(np.asarray(text_features, dtype=np.float32))
    BB, DD = im.shape
    R = BB // W

    # ---- kernel A: per-shard stats ------------------------------------
    ncA = _get("A", BB, DD, W)
    mapsA = []
    for c in range(W):
        mapsA.append({
            "imTs": np.ascontiguousarray(im[c * R:(c + 1) * R].T),
            "txTs": np.ascontiguousarray(tx[c * R:(c + 1) * R].T),
        })
    resA = _run(ncA, mapsA, "A")
    decoded = [_decode_stats_block(r["stats"]) for r in resA.results]
    inv_im_full = np.concatenate([d[0] for d in decoded])
    inv_tx_full = np.concatenate([d[1] for d in decoded])
    mb_full = np.concatenate([d[2] for d in decoded])
    nmb_full = np.concatenate([d[3] for d in decoded])

    # ---- kernel B: sim blocks + loss ----------------------------------
    RI, RJ = BB // GR, BB // GS
    ncB = _get("B", BB, DD)
    mapsB = []
    txT_parts = [np.ascontiguousarray(tx[s * RJ:(s + 1) * RJ].T)
                 for s in range(GS)]
    imT_parts = [np.ascontiguousarray(im[r * RI:(r + 1) * RI].T)
                 for r in range(GR)]
    for c in range(GR * GS):
        r, s = c // GS, c % GS
        mapsB.append({
            "imT": imT_parts[r],
            "txT": txT_parts[s],
            "invim_col": np.ascontiguousarray(
                inv_im_full[r * RI:(r + 1) * RI].reshape(-1, 128).T),
            "mb_col": np.ascontiguousarray(
                mb_full[r * RI:(r + 1) * RI].reshape(-1, 128).T),
            "invtx_row": inv_tx_full[s * RJ:(s + 1) * RJ].reshape(
                1, -1).astype(ml_dtypes.bfloat16),
            "nc2_row": nmb_full[s * RJ:(s + 1) * RJ].reshape(
                1, -1).astype(ml_dtypes.bfloat16),
        })
    resB = _run(ncB, mapsB, "B")

    sim = np.empty((BB, BB), dtype=np.float32)
    loss = 0.0
    for c in range(GR * GS):
        r, s = c // GS, c % GS
        sim[r * RI:(r + 1) * RI, s * RJ:(s + 1) * RJ] = resB.results[c]["sim"]
        loss += float(resB.results[c]["loss"][0, 0])
    return np.float32(loss), sim


# revision 8
# speedup vs baseline: 1.0766x; 1.0002x over previous
"""VSE contrastive hinge loss + similarity matrix on 8 TRN2 NeuronCores.

Two-NEFF pipeline (collectives cost ~80us of barrier/trigger overhead in this
environment, measured, so the cross-core exchange goes through the host):

  Kernel A (stats): core c owns row-block c (512 rows) of BOTH feature
  matrices; computes inv_norm(im), inv_norm(tx) and the diagonal
  d = cos(im_c, tx_c); outputs {inv_im, inv_tx, M-d, d-M} in a [128, 16]
  column-tile layout (out[p, 4*g + t] = vec_g[t*128 + p]).

  Host: concatenates the 8 stat blocks and re-slices them per kernel-B core
  (pure data movement).

  Kernel B (main): 4x2 grid - core c=(r,s) owns im rows I_r (1024) x tx rows
  J_s (2048). Computes raw im @ tx^T with bf16 matmuls (tx pre-scaled by
  inv_tx during the f32->bf16 cast), row-scales by inv_im on PSUM eviction,
  writes its [1024, 2048] f32 sim block, and accumulates both hinge sums:
    r1 = sum relu(sim + (M - d_i))     (ACT relu + accum, per-row bias)
    r2 = sum max(sim, d_j - M)         (DVE max + reduce, corrected by
                                        - rows * sum_j (d_j - M))
  minus 2*M*512 per core for the diagonal cells (summed over the 8 cores
  this equals the reference's diagonal mask up to fp rounding).
"""

import sys

sys.path.insert(0, "/opt/trn_rl_repo")

import ml_dtypes
import numpy as np

import concourse.bass as bass
import concourse.bacc as bacc
import concourse.mybir as mybir
import concourse.tile as tile

B = 4096
D = 1024
W = 8
GR = 4            # im row blocks (kernel B grid)
GS = 2            # tx row blocks
MARGIN = 0.2
EPS = 1e-12

F32 = mybir.dt.float32
BF16 = mybir.dt.bfloat16
AF = mybir.ActivationFunctionType
ALU = mybir.AluOpType
AX = mybir.AxisListType


def build_stats(BB=B, DD=D, WW=W):
    """Kernel A: per-shard norms + diagonal, outputs in [128, 4G] layout."""
    R = BB // WW          # 512
    KT = DD // 128        # 8
    TT = R // 128         # 4 columns per quantity

    nc = bacc.Bacc("TRN2", target_bir_lowering=False, debug=False,
                   num_devices=WW)
    imTs = nc.dram_tensor("imTs", [DD, R], F32, kind="ExternalInput")
    txTs = nc.dram_tensor("txTs", [DD, R], F32, kind="ExternalInput")
    stats_out = nc.dram_tensor("stats", [128, 4 * TT], F32,
                               kind="ExternalOutput")

    with tile.TileContext(nc) as tc:
        with (
            tc.tile_pool(name="res", bufs=1) as res,
            tc.tile_pool(name="stage", bufs=2) as stage,
            tc.tile_pool(name="scr", bufs=3) as scr,
            tc.tile_pool(name="dram", bufs=1, space="DRAM") as dram,
            tc.tile_pool(name="ppool", bufs=1, space="PSUM") as ppool,
        ):
            ones_bf = res.tile([128, 1], BF16, name="ones_bf")
            nc.vector.memset(ones_bf[:], 1.0)

            # quarter-sized loads so per-kt stats start as tiles land
            HK = max(KT // 4, 1)
            NH = KT // HK
            im_st = [stage.tile([128, HK, R], F32, name="im_st", tag="ist",
                                bufs=NH)
                     for _ in range(NH)]
            tx_st = [stage.tile([128, HK, R], F32, name="tx_st", tag="tst",
                                bufs=NH)
                     for _ in range(NH)]
            for h in range(NH):
                src = imTs[h * HK * 128:(h + 1) * HK * 128, :]
                nc.sync.dma_start(
                    im_st[h][:], src.rearrange("(k p) r -> p k r", p=128))
                src = txTs[h * HK * 128:(h + 1) * HK * 128, :]
                nc.sync.dma_start(
                    tx_st[h][:], src.rearrange("(k p) r -> p k r", p=128))

            ps_imn = ppool.tile([1, R], F32, name="ps_imn")
            ps_txn = ppool.tile([1, R], F32, name="ps_txn")
            ps_dg = ppool.tile([1, R], F32, name="ps_dg")
            # stats work three-way split across ACT / DVE / GpSimd so no
            # single engine trails the loads
            for kt in range(KT):
                h, k = divmod(kt, HK)
                ims = im_st[h][:, k, :]
                txs = tx_st[h][:, k, :]
                sq_i = scr.tile([128, R], BF16, name="sq_i")
                nc.scalar.activation(sq_i[:], ims, AF.Square)
                nc.tensor.matmul(ps_imn[:], ones_bf[:], sq_i[:],
                                 start=(kt == 0), stop=(kt == KT - 1))
                sq_t = scr.tile([128, R], BF16, name="sq_t")
                nc.vector.tensor_mul(sq_t[:], txs, txs)
                nc.tensor.matmul(ps_txn[:], ones_bf[:], sq_t[:],
                                 start=(kt == 0), stop=(kt == KT - 1))
                pr = scr.tile([128, R], BF16, name="pr")
                if kt % 2 == 0:
                    nc.vector.tensor_mul(pr[:], ims, txs)
                else:
                    nc.gpsimd.tensor_mul(pr[:], ims, txs)
                nc.tensor.matmul(ps_dg[:], ones_bf[:], pr[:],
                                 start=(kt == 0), stop=(kt == KT - 1))

            # evict the three [1, R] rows, bounce through DRAM into a
            # [128, 3*TT] column layout so the inverse chain runs wide
            row3 = res.tile([1, 3 * R], F32, name="row3")
            nc.scalar.copy(row3[0:1, 0:R], ps_imn[:])
            nc.scalar.copy(row3[0:1, R:2 * R], ps_txn[:])
            nc.scalar.copy(row3[0:1, 2 * R:3 * R], ps_dg[:])
            bounce = dram.tile([1, 3 * R], F32, name="bounce")
            nc.sync.dma_start(bounce[:], row3[:])
            nst = res.tile([128, 3 * TT], F32, name="nst")
            nc.sync.dma_start(
                nst[:], bounce[0:1, :].rearrange("o (g t p) -> (o p) (g t)",
                                                 p=128, g=3))

            # cols [0:TT]=sum im^2, [TT:2TT]=sum tx^2, [2TT:3TT]=im.tx
            nrm = nst[:, 0:2 * TT]
            nc.scalar.activation(nrm, nrm, AF.Sqrt)
            nc.vector.tensor_scalar_max(nrm, nrm, EPS)
            nc.vector.reciprocal(nrm, nrm)
            dg = nst[:, 2 * TT:3 * TT]
            nc.vector.tensor_mul(dg, dg, nst[:, 0:TT])
            nc.vector.tensor_mul(dg, dg, nst[:, TT:2 * TT])

            out_t = res.tile([128, 4 * TT], F32, name="out_t")
            nc.vector.tensor_copy(out_t[:, 0:2 * TT], nst[:, 0:2 * TT])
            # M - d and d - M
            nc.scalar.activation(out_t[:, 2 * TT:3 * TT], dg, AF.Copy,
                                 bias=MARGIN, scale=-1.0)
            nc.vector.tensor_scalar_sub(out_t[:, 3 * TT:4 * TT], dg, MARGIN)
            nc.sync.dma_start(stats_out[:], out_t[:])

    nc.compile()
    return nc


def build_main(BB=B, DD=D):
    """Kernel B: sim block + hinge loss on the 4x2 grid."""
    RI = BB // GR          # 1024 im rows per core
    RJ = BB // GS          # 2048 tx rows per core
    KT = DD // 128         # 8
    TT = RI // 128         # 8
    NCH = RJ // 512        # 4

    nc = bacc.Bacc("TRN2", target_bir_lowering=False, debug=False,
                   num_devices=GR * GS)
    imT = nc.dram_tensor("imT", [DD, RI], F32, kind="ExternalInput")
    txT = nc.dram_tensor("txT", [DD, RJ], F32, kind="ExternalInput")
    invim_col = nc.dram_tensor("invim_col", [128, TT], F32,
                               kind="ExternalInput")
    mb_col = nc.dram_tensor("mb_col", [128, TT], F32, kind="ExternalInput")
    invtx_row = nc.dram_tensor("invtx_row", [1, RJ], BF16,
                               kind="ExternalInput")
    nc2_row = nc.dram_tensor("nc2_row", [1, RJ], BF16, kind="ExternalInput")
    sim_out = nc.dram_tensor("sim", [RI, RJ], F32, kind="ExternalOutput")
    loss_out = nc.dram_tensor("loss", [1, 1], F32, kind="ExternalOutput")

    with tile.TileContext(nc) as tc:
        with (
            tc.tile_pool(name="res", bufs=1) as res,
            tc.tile_pool(name="stage", bufs=2) as stage,
            tc.tile_pool(name="mch", bufs=4) as mch,
            tc.tile_pool(name="mscr", bufs=3) as mscr,
        ):
            # order matters: the matmul-feeding loads go first, the
            # loss-only tensors (mbc, nc2bc) after them
            ivc = res.tile([128, TT], F32, name="ivc")
            nc.sync.dma_start(ivc[:], invim_col[:])
            invtx_bc = res.tile([128, RJ], BF16, name="invtx_bc")
            nc.sync.dma_start(invtx_bc[:],
                              invtx_row[0:1, :].to_broadcast((128, RJ)))

            # loads: im as 2 big DMAs, tx as 4; casts consume stage slices.
            # tx gets pre-scaled by inv_tx during the cast.
            im_bf = res.tile([128, KT, RI], BF16, name="im_bf")
            tx_bf = res.tile([128, KT, RJ], BF16, name="tx_bf")
            HK = KT // 2
            im_st = [stage.tile([128, HK, RI], F32, name="im_st", tag="ist")
                     for _ in range(2)]
            tx_st = [stage.tile([128, RJ], F32, name="tx_st", tag="tst",
                                bufs=3)
                     for _ in range(KT)]
            nc.sync.dma_start(tx_st[0][:], txT[0:128, :])
            nc.sync.dma_start(tx_st[1][:], txT[128:256, :])
            nc.sync.dma_start(
                im_st[0][:],
                imT[0:HK * 128, :].rearrange("(k p) r -> p k r", p=128))
            nc.sync.dma_start(tx_st[2][:], txT[256:384, :])
            nc.sync.dma_start(tx_st[3][:], txT[384:512, :])
            nc.sync.dma_start(
                im_st[1][:],
                imT[HK * 128:KT * 128, :].rearrange("(k p) r -> p k r",
                                                    p=128))
            for kt in range(4, KT):
                nc.sync.dma_start(tx_st[kt][:],
                                  txT[kt * 128:(kt + 1) * 128, :])

            mbc = res.tile([128, TT], F32, name="mbc")
            nc.sync.dma_start(mbc[:], mb_col[:])
            nc2bc = res.tile([128, RJ], BF16, name="nc2bc")
            nc.sync.dma_start(nc2bc[:],
                              nc2_row[0:1, :].to_broadcast((128, RJ)))
            cs = res.tile([1, 1], F32, name="cs")
            nc.vector.reduce_sum(cs[:], nc2bc[0:1, :], axis=AX.X)
            ones_f = res.tile([128, 1], F32, name="ones_f")
            nc.vector.memset(ones_f[:], 1.0)
            zbias = res.tile([128, 1], F32, name="zbias")
            nc.vector.memset(zbias[:], 0.0)

            for kt in range(KT):
                nc.vector.tensor_mul(tx_bf[:, kt, :], tx_st[kt][:],
                                     invtx_bc[:])
            for kt in range(KT):
                h, k = divmod(kt, HK)
                nc.scalar.copy(im_bf[:, kt, :], im_st[h][:, k, :])

            slots1 = res.tile([128, TT], F32, name="slots1")
            slots2 = res.tile([128, TT], F32, name="slots2")

            with tc.tile_pool(name="mpsum", bufs=8, space="PSUM") as mpsum:
                for t in range(TT):
                    pss = [mpsum.tile([128, 512], F32, name="ps")
                           for _ in range(NCH)]
                    for kt in range(KT):
                        for n in range(NCH):
                            nc.tensor.matmul(
                                pss[n][:],
                                im_bf[:, kt, t * 128:(t + 1) * 128],
                                tx_bf[:, kt, n * 512:(n + 1) * 512],
                                start=(kt == 0), stop=(kt == KT - 1))
                    simf = mch.tile([128, RJ], F32, name="simf")
                    for n in range(NCH):
                        dst = simf[:, n * 512:(n + 1) * 512]
                        if n % 2 == 0:
                            nc.scalar.activation(dst, pss[n][:], AF.Copy,
                                                 scale=ivc[:, t:t + 1])
                        else:
                            nc.vector.tensor_scalar_mul(dst, pss[n][:],
                                                        ivc[:, t:t + 1])
                    nc.gpsimd.dma_start(sim_out[t * 128:(t + 1) * 128, :],
                                        simf[:])
                    # caption-negative hinge
                    sc1 = mscr.tile([128, RJ], BF16, name="sc1")
                    nc.scalar.activation(sc1[:], simf[:], AF.Relu,
                                         bias=mbc[:, t:t + 1],
                                         accum_out=slots1[:, t:t + 1])
                    # image-negative hinge via max(sim, d_j - M); the row
                    # sum alternates DVE reduce / ACT accumulate to balance
                    mx = mscr.tile([128, RJ], BF16, name="mx")
                    nc.vector.tensor_max(mx[:], simf[:], nc2bc[:])
                    if t % 2 == 0:
                        nc.vector.reduce_sum(slots2[:, t:t + 1], mx[:],
                                             axis=AX.X)
                    else:
                        jk = mscr.tile([128, RJ], BF16, name="jk")
                        nc.scalar.activation(jk[:], mx[:], AF.Identity,
                                             bias=zbias[:],
                                             accum_out=slots2[:, t:t + 1])

            red1 = res.tile([128, 1], F32, name="red1")
            nc.vector.reduce_sum(red1[:], slots1[:], axis=AX.X)
            red2 = res.tile([128, 1], F32, name="red2")
            nc.vector.reduce_sum(red2[:], slots2[:], axis=AX.X)
            nc.vector.tensor_add(red1[:], red1[:], red2[:])
            lf = res.tile([1, 1], F32, name="lf")
            with tc.tile_pool(name="lpsum", bufs=1, space="PSUM") as lpsum:
                ps_l = lpsum.tile([1, 1], F32, name="ps_l")
                nc.tensor.matmul(ps_l[:], red1[:], ones_f[:], start=True,
                                 stop=True)
                nc.scalar.copy(lf[:], ps_l[:])
            # r2 correction: subtract RI * sum_j (d_j - M)
            nc.vector.tensor_scalar_mul(cs[:], cs[:], float(RI))
            nc.vector.tensor_sub(lf[:], lf[:], cs[:])
            # diagonal cells: minus 2*M*(B/8) per core, summing to 2*M*B
            nc.vector.tensor_scalar_sub(
                lf[:], lf[:], float(2.0 * MARGIN * BB / (GR * GS)))
            nc.sync.dma_start(loss_out[0:1, 0:1], lf[:])

    nc.compile()
    return nc


_CACHE = {}

# test-harness knobs: when TRACE is set, each NEFF run is profiled and its
# exec_time_ns is appended to LAST_EXEC_NS
TRACE = False
LAST_EXEC_NS = []


def _get(which, *args):
    key = (which, args)
    if key not in _CACHE:
        _CACHE[key] = (build_stats(*args) if which == "A"
                       else build_main(*args))
    return _CACHE[key]


def _run(nc, maps, tag):
    from concourse.bass_utils import run_bass_kernel_spmd

    if not TRACE:
        return run_bass_kernel_spmd(nc, maps, core_ids=list(range(len(maps))))
    import os
    import shutil
    tmpdir = f"/tmp/trn_trace_{tag}"
    shutil.rmtree(tmpdir, ignore_errors=True)
    os.makedirs(tmpdir)
    res = run_bass_kernel_spmd(nc, maps, core_ids=list(range(len(maps))),
                               trace=True, tmpdir=tmpdir)
    LAST_EXEC_NS.append((tag, res.exec_time_ns, tmpdir))
    return res


def _decode_stats_block(block):
    """[128, 4*TT] column-tile layout -> four [R] vectors."""
    TT = block.shape[1] // 4
    return [block[:, g * TT:(g + 1) * TT].T.reshape(-1) for g in range(4)]


def kernel(image_features, text_features):
    im = np.ascontiguousarray(np.asarray(image_features, dtype=np.float32))
    tx = np.ascontiguousarray(np.asarray(text_features, dtype=np.float32))
    BB, DD = im.shape
    R = BB // W

    # ---- kernel A: per-shard stats ------------------------------------
    ncA = _get("A", BB, DD, W)
    mapsA = []
    for c in range(W):
        mapsA.append({
            "imTs": np.ascontiguousarray(im[c * R:(c + 1) * R].T),
            "txTs": np.ascontiguousarray(tx[c * R:(c + 1) * R].T),
        })
    resA = _run(ncA, mapsA, "A")
    decoded = [_decode_stats_block(r["stats"]) for r in resA.results]
    inv_im_full = np.concatenate([d[0] for d in decoded])
    inv_tx_full = np.concatenate([d[1] for d in decoded])
    mb_full = np.concatenate([d[2] for d in decoded])
    nmb_full = np.concatenate([d[3] for d in decoded])

    # ---- kernel B: sim blocks + loss ----------------------------------
    RI, RJ = BB // GR, BB // GS
    ncB = _get("B", BB, DD)
    mapsB = []
    txT_parts = [np.ascontiguousarray(tx[s * RJ:(s + 1) * RJ].T)
                 for s in range(GS)]
    imT_parts = [np.ascontiguousarray(im[r * RI:(r + 1) * RI].T)
                 for r in range(GR)]
    for c in range(GR * GS):
        r, s = c // GS, c % GS
        mapsB.append({
            "imT": imT_parts[r],
            "txT": txT_parts[s],
            "invim_col": np.ascontiguousarray(
                inv_im_full[r * RI:(r + 1) * RI].reshape(-1, 128).T),
            "mb_col": np.ascontiguousarray(
                mb_full[r * RI:(r + 1) * RI].reshape(-1, 128).T),
            "invtx_row": inv_tx_full[s * RJ:(s + 1) * RJ].reshape(
                1, -1).astype(ml_dtypes.bfloat16),
            "nc2_row": nmb_full[s * RJ:(s + 1) * RJ].reshape(
                1, -1).astype(ml_dtypes.bfloat16),
        })
    resB = _run(ncB, mapsB, "B")

    sim = np.empty((BB, BB), dtype=np.float32)
    loss = 0.0
    for c in range(GR * GS):
        r, s = c // GS, c % GS
        sim[r * RI:(r + 1) * RI, s * RJ:(s + 1) * RJ] = resB.results[c]["sim"]
        loss += float(resB.results[c]["loss"][0, 0])
    return np.float32(loss), sim
